# revision 5
# baseline (speedup 1.0000x reference)
"""AttentionCritic forward on 8 TRN2 NeuronCores.

Data-parallel: batch 4096 sharded 512/core, weights replicated, no
collectives. On-chip layout is feature-major ([feature, batch] tiles of
[128, 512]) so the whole matmul chain runs without transposes and biases
are per-partition. softmax over 2 logits is computed as
sigmoid(logit1 - logit0) with the weight difference folded on the host.

COMPUTE selects the matmul operand dtype: "f32" (safe) or "bf16"
(4x PE throughput, half the weight DMA; PSUM accumulation and both
outputs stay f32).
"""

import os
import sys

import numpy as np

for _p in ("/root/.axon_site/_ro/trn_rl_repo", "/opt/trn_rl_repo"):
    if _p not in sys.path:
        sys.path.append(_p)

B = 4096
D = 1024  # obs dim
E = 512  # e_outputs
H = 2048  # H1 == H2
NCORES = 8
BL = B // NCORES  # 512 local batch
P = 128

KD = D // P  # 8
ME = E // P  # 4
KE = E // P  # 4
MD = D // P  # 8
MH = H // P  # 16
KH = H // P  # 16

COMPUTE = os.environ.get("BASS_KERNEL_COMPUTE", "f32")

_cache = {}

# Filled by the last kernel() call for the test harness.
LAST_RESULTS = None


def _np_compute_dtype():
    if COMPUTE == "bf16":
        import ml_dtypes

        return ml_dtypes.bfloat16
    return np.float32


def _build_nc():
    import concourse.bacc as bacc
    import concourse.tile as tile
    from concourse import mybir

    f32 = mybir.dt.float32
    cdt = mybir.dt.bfloat16 if COMPUTE == "bf16" else f32
    AFT = mybir.ActivationFunctionType

    nc = bacc.Bacc(None, target_bir_lowering=False, debug=False)

    obsT_d = nc.declare_dram_parameter("obsT", [P, KD * BL], cdt, isOutput=False)
    WeT_d = nc.declare_dram_parameter("WeT", [P, KD * E], cdt, isOutput=False)
    beP_d = nc.declare_dram_parameter("beP", [P, ME], f32, isOutput=False)
    WdT_d = nc.declare_dram_parameter("WdT", [P, KE * D], cdt, isOutput=False)
    bdP_d = nc.declare_dram_parameter("bdP", [P, MD], f32, isOutput=False)
    W1c_d = nc.declare_dram_parameter("W1c", [MH, P, KD * P], cdt, isOutput=False)
    b1P_d = nc.declare_dram_parameter("b1P", [P, MH], f32, isOutput=False)
    W2c_d = nc.declare_dram_parameter("W2c", [MH, P, KH * P], cdt, isOutput=False)
    b2P_d = nc.declare_dram_parameter("b2P", [P, MH], f32, isOutput=False)
    w3P_d = nc.declare_dram_parameter("w3P", [P, KH], cdt, isOutput=False)
    b3s_d = nc.declare_dram_parameter("b3s", [1, 1], f32, isOutput=False)

    attnT_d = nc.declare_dram_parameter("attnT", [P, MD * BL], f32, isOutput=True)
    vout_d = nc.declare_dram_parameter("vout", [1, BL], f32, isOutput=True)

    with tile.TileContext(nc) as tc:
        with (
            tc.tile_pool(name="res", bufs=1) as res,
            tc.tile_pool(name="acts", bufs=1) as acts,
            tc.tile_pool(name="wstream", bufs=4) as wstream,
            tc.tile_pool(name="psum", bufs=4, space="PSUM") as psum,
        ):
            # resident inputs
            obsT = res.tile([P, KD * BL], cdt, tag="obsT")
            WeT = res.tile([P, KD * E], cdt, tag="WeT")
            WdT = res.tile([P, KE * D], cdt, tag="WdT")
            beP = res.tile([P, ME], f32, tag="beP")
            bdP = res.tile([P, MD], f32, tag="bdP")
            b1P = res.tile([P, MH], f32, tag="b1P")
            b2P = res.tile([P, MH], f32, tag="b2P")
            w3P = res.tile([P, KH], cdt, tag="w3P")
            b3s = res.tile([1, 1], f32, tag="b3s")

            # resident activations (feature-major, batch on the free dim)
            eT = acts.tile([P, ME * BL], cdt, tag="eT")
            aT = acts.tile([P, MD * BL], f32, tag="aT")
            sT = acts.tile([P, KD * BL], cdt, tag="sT")
            h1T = acts.tile([P, MH * BL], cdt, tag="h1T")
            h2T = acts.tile([P, MH * BL], cdt, tag="h2T")
            vT = acts.tile([1, BL], f32, tag="vT")

            # loads; per-k-tile DMAs to spread over queues
            for k in range(KD):
                nc.sync.dma_start(
                    out=obsT[:, k * BL : (k + 1) * BL],
                    in_=obsT_d[:, k * BL : (k + 1) * BL],
                )
                nc.sync.dma_start(
                    out=WeT[:, k * E : (k + 1) * E],
                    in_=WeT_d[:, k * E : (k + 1) * E],
                )
            for k in range(KE):
                nc.sync.dma_start(
                    out=WdT[:, k * D : (k + 1) * D],
                    in_=WdT_d[:, k * D : (k + 1) * D],
                )
            nc.sync.dma_start(out=beP[:, :], in_=beP_d[:, :])
            nc.sync.dma_start(out=bdP[:, :], in_=bdP_d[:, :])
            nc.sync.dma_start(out=b1P[:, :], in_=b1P_d[:, :])
            nc.sync.dma_start(out=b2P[:, :], in_=b2P_d[:, :])
            nc.sync.dma_start(out=w3P[:, :], in_=w3P_d[:, :])
            nc.sync.dma_start(out=b3s[:, :], in_=b3s_d[:, :])

            # L1: eT = tanh(We @ obsT + be), [512, BL]
            for m in range(ME):
                ps = psum.tile([P, BL], f32, tag="ps")
                for k in range(KD):
                    nc.tensor.matmul(
                        ps[:, :],
                        lhsT=WeT[:, k * E + m * P : k * E + (m + 1) * P],
                        rhs=obsT[:, k * BL : (k + 1) * BL],
                        start=(k == 0),
                        stop=(k == KD - 1),
                    )
                nc.scalar.activation(
                    eT[:, m * BL : (m + 1) * BL],
                    ps[:, :],
                    AFT.Tanh,
                    bias=beP[:, m : m + 1],
                )

            # L2: aT = sigmoid(Wd @ eT + bd), [1024, BL]; then
            # sT = obsT * aT, and DMA the attention out.
            for m in range(MD):
                ps = psum.tile([P, BL], f32, tag="ps")
                for k in range(KE):
                    nc.tensor.matmul(
                        ps[:, :],
                        lhsT=WdT[:, k * D + m * P : k * D + (m + 1) * P],
                        rhs=eT[:, k * BL : (k + 1) * BL],
                        start=(k == 0),
                        stop=(k == KE - 1),
                    )
                nc.scalar.activation(
                    aT[:, m * BL : (m + 1) * BL],
                    ps[:, :],
                    AFT.Sigmoid,
                    bias=bdP[:, m : m + 1],
                )
                nc.sync.dma_start(
                    out=attnT_d[:, m * BL : (m + 1) * BL],
                    in_=aT[:, m * BL : (m + 1) * BL],
                )
                nc.vector.tensor_mul(
                    sT[:, m * BL : (m + 1) * BL],
                    obsT[:, m * BL : (m + 1) * BL],
                    aT[:, m * BL : (m + 1) * BL],
                )

            # L3: h1T = tanh(W1 @ sT + b1), [2048, BL]; W1 streamed per m-chunk
            for m in range(MH):
                wc = wstream.tile([P, KD * P], cdt, tag="wchunk", name=f"w1c{m}")
                nc.sync.dma_start(out=wc[:, :], in_=W1c_d[m, :, :])
                ps = psum.tile([P, BL], f32, tag="ps")
                for k in range(KD):
                    nc.tensor.matmul(
                        ps[:, :],
                        lhsT=wc[:, k * P : (k + 1) * P],
                        rhs=sT[:, k * BL : (k + 1) * BL],
                        start=(k == 0),
                        stop=(k == KD - 1),
                    )
                nc.scalar.activation(
                    h1T[:, m * BL : (m + 1) * BL],
                    ps[:, :],
                    AFT.Tanh,
                    bias=b1P[:, m : m + 1],
                )

            # L4: h2T = tanh(W2 @ h1T + b2), [2048, BL]; W2 streamed per m-chunk
            for m in range(MH):
                wc = wstream.tile([P, KH * P], cdt, tag="wchunk", name=f"w2c{m}")
                nc.sync.dma_start(out=wc[:, :], in_=W2c_d[m, :, :])
                ps = psum.tile([P, BL], f32, tag="ps")
                for k in range(KH):
                    nc.tensor.matmul(
                        ps[:, :],
                        lhsT=wc[:, k * P : (k + 1) * P],
                        rhs=h1T[:, k * BL : (k + 1) * BL],
                        start=(k == 0),
                        stop=(k == KH - 1),
                    )
                nc.scalar.activation(
                    h2T[:, m * BL : (m + 1) * BL],
                    ps[:, :],
                    AFT.Tanh,
                    bias=b2P[:, m : m + 1],
                )

            # L5: v = W3 @ h2T + b3, [1, BL]
            psv = psum.tile([P, BL], f32, tag="ps", name="psv")
            for k in range(KH):
                nc.tensor.matmul(
                    psv[:1, :],
                    lhsT=w3P[:, k : k + 1],
                    rhs=h2T[:, k * BL : (k + 1) * BL],
                    start=(k == 0),
                    stop=(k == KH - 1),
                )
            nc.vector.tensor_scalar_add(vT[:1, :], psv[:1, :], b3s[:1, :])
            nc.sync.dma_start(out=vout_d[:, :], in_=vT[:1, :])

    nc.finalize()
    return nc


def _pack_shared(We, be, attn_W, attn_b, W1, b1, W2, b2, W3, b3):
    f = np.float32
    c = _np_compute_dtype()
    WeT = np.ascontiguousarray(
        We.T.reshape(KD, P, E).transpose(1, 0, 2).reshape(P, KD * E)
    ).astype(c)
    beP = np.ascontiguousarray(be.reshape(ME, P).T, dtype=f)
    Wd = attn_W[:, :, 1] - attn_W[:, :, 0]  # [D, E]
    WdT = np.ascontiguousarray(
        Wd.T.reshape(KE, P, D).transpose(1, 0, 2).reshape(P, KE * D)
    ).astype(c)
    bd = attn_b[:, 1] - attn_b[:, 0]
    bdP = np.ascontiguousarray(bd.reshape(MD, P).T, dtype=f)
    W1c = np.ascontiguousarray(
        W1.reshape(MH, P, KD, P).transpose(0, 3, 2, 1).reshape(MH, P, KD * P)
    ).astype(c)
    b1P = np.ascontiguousarray(b1.reshape(MH, P).T, dtype=f)
    W2c = np.ascontiguousarray(
        W2.reshape(MH, P, KH, P).transpose(0, 3, 2, 1).reshape(MH, P, KH * P)
    ).astype(c)
    b2P = np.ascontiguousarray(b2.reshape(MH, P).T, dtype=f)
    w3P = np.ascontiguousarray(W3.reshape(KH, P).T).astype(c)
    b3s = np.asarray(b3, dtype=f).reshape(1, 1)
    return {
        "WeT": WeT,
        "beP": beP,
        "WdT": WdT,
        "bdP": bdP,
        "W1c": W1c,
        "b1P": b1P,
        "W2c": W2c,
        "b2P": b2P,
        "w3P": w3P,
        "b3s": b3s,
    }


def kernel(obs, We, be, attn_W, attn_b, W1, b1, W2, b2, W3, b3):
    global LAST_RESULTS

    from concourse.bass_utils import run_bass_kernel_spmd

    if "nc" not in _cache:
        _cache["nc"] = _build_nc()
    nc = _cache["nc"]

    shared = _pack_shared(We, be, attn_W, attn_b, W1, b1, W2, b2, W3, b3)
    cdt = _np_compute_dtype()
    in_maps = []
    for c in range(NCORES):
        sh = np.asarray(obs[c * BL : (c + 1) * BL], dtype=np.float32)
        obsT = np.ascontiguousarray(
            sh.T.reshape(KD, P, BL).transpose(1, 0, 2).reshape(P, KD * BL)
        ).astype(cdt)
        in_maps.append({"obsT": obsT, **shared})

    trace = bool(os.environ.get("BASS_KERNEL_TRACE"))
    out = run_bass_kernel_spmd(
        nc,
        in_maps,
        core_ids=list(range(NCORES)),
        trace=trace,
    )
    LAST_RESULTS = out

    attn = np.empty((B, D), dtype=np.float32)
    v = np.empty((B,), dtype=np.float32)
    for c in range(NCORES):
        r = out.results[c]
        a = r["attnT"].reshape(P, MD, BL).transpose(2, 1, 0).reshape(BL, D)
        attn[c * BL : (c + 1) * BL] = a
        v[c * BL : (c + 1) * BL] = r["vout"][0]
    return v, attn
